# revision 1
# baseline (speedup 1.0000x reference)
"""Trainium2 Bass kernel for nn_LpAlignEntropyLoss.

Loss over three views z1,z2,z3 (each [8192,128] f32):
  for each pair (i<j):
    pos += mean_m ||zi_m - zj_m||
    neg += mean_m [ ln(sum_n exp(-d_mn)) - ln(B) ],  d = cdist(zi, zj)
  loss = (0.5*pos + 0.5*neg) / 3

Strategy: shard the 8192 rows across 8 cores (1024 each). Each core holds
all three views transposed ([128=D, 8192=B]) in SBUF as bf16, computes its
row-block of each pairwise squared-distance matrix with PE matmuls: the
-2*dot term is the main bf16 matmul, the +b2[n] column-norm term is folded
in as a K=1 accumulating matmul, and the +a2[m] row-norm term rides the
ACT bias. ScalarE then does sqrt (PSUM->SBUF fp16) and exp(16-d) with a
fused row-accumulate, batched per m-block by activation-table set to
bound table-switch cost. Host sums the 8 partial scalars; no collectives.
"""

import math

import numpy as np
import ml_dtypes

import concourse.bacc as bacc
import concourse.bass as bass
import concourse.mybir as mybir
import concourse.tile as tile
from concourse.tile import add_dep_helper
from concourse.bass_utils import run_bass_kernel_spmd

B, D = 8192, 128
NCORES = 8
ML = B // NCORES          # rows per core (1024)
MB = ML // 128            # m-blocks per core (8)
NCHUNK = 2048             # psum chunk (4 banks)
NQ = B // NCHUNK          # chunks per row (4)
PAIRS = [(0, 1), (0, 2), (1, 2)]
TAU = 1.0
ALPHA = 0.5
B2_CENTER = 128.0         # E[||z||^2] for z~N(0,I_128); centers the bf16 bias row
EXP_SHIFT = 16.0          # e^(SHIFT-d) keeps fp16 in range for d in [9, 27]

F32 = mybir.dt.float32
BF16 = mybir.dt.bfloat16
FP16 = mybir.dt.float16
AF = mybir.ActivationFunctionType
ALU = mybir.AluOpType
AX = mybir.AxisListType



def build(nc: bacc.Bacc):
    zt = [nc.dram_tensor(f"zt{v}", [D, B], BF16, kind="ExternalInput") for v in range(3)]
    blk = [nc.dram_tensor(f"blk{v}", [D, ML], BF16, kind="ExternalInput") for v in range(3)]
    b2h_in = {j: nc.dram_tensor(f"b2h{j}", [1, B], BF16, kind="ExternalInput")
              for j in sorted({j for _, j in PAIRS})}
    biasp_in = nc.dram_tensor("biaspall", [128, 3 * MB], F32, kind="ExternalInput")
    sqpos_in = nc.dram_tensor("sqposall", [128, 3 * MB], F32, kind="ExternalInput")
    out = nc.dram_tensor("out", [2, 1], F32, kind="ExternalOutput")

    rhs_views = sorted({j for _, j in PAIRS})  # views used as columns (1, 2)

    with tile.TileContext(nc) as tc:
        with tc.tile_pool(name="persist", bufs=1) as persist:
            # ---- persistent SBUF ----
            rhs_views_l = sorted({j for _, j in PAIRS})
            ztc = {j: [persist.tile([D, NCHUNK], BF16, tag=f"ztc{j}_{q}", name=f"ztc{j}_{q}")
                       for q in range(NQ)] for j in rhs_views_l}
            blks = [persist.tile([D, ML], BF16, tag=f"blks{v}", name=f"blks{v}") for v in range(3)]
            b2half = {j: persist.tile([1, B], BF16, tag=f"b2h{j}", name=f"b2h{j}") for j in rhs_views_l}
            biasp_t = persist.tile([128, 3 * MB], F32, tag="biaspall")
            sqpos = persist.tile([128, 3 * MB], F32, tag="sqposall")
            # small norm/bias inputs first: the first bias matmul gates on them
            for j in rhs_views_l:
                nc.sync.dma_start(b2half[j][:], b2h_in[j][:])
            nc.sync.dma_start(biasp_t[:], biasp_in[:])
            nc.sync.dma_start(sqpos[:], sqpos_in[:])
            nc.sync.dma_start(ztc[1][0][:], zt[1][:, 0:NCHUNK])
            nc.sync.dma_start(blks[0][:], blk[0][:])
            nc.sync.dma_start(ztc[2][0][:], zt[2][:, 0:NCHUNK])
            nc.sync.dma_start(blks[1][:], blk[1][:])
            nc.sync.dma_start(blks[2][:], blk[2][:])
            for q in range(1, NQ):
                for j in rhs_views_l:
                    nc.sync.dma_start(ztc[j][q][:], zt[j][:, q * NCHUNK:(q + 1) * NCHUNK])

            ones_bf_row = persist.tile([1, 128], BF16, tag="ones_bf_row")
            nc.vector.memset(ones_bf_row[:], 1.0)
            ones_f32_col = persist.tile([128, 1], F32, tag="ones_f32_col")
            nc.vector.memset(ones_f32_col[:], 1.0)
            shift16 = persist.tile([128, 1], F32, tag="shift16")
            nc.vector.memset(shift16[:], EXP_SHIFT)

            # host-computed norm/bias prep (from the same rounded bf16 z):
            # b2half[j][n] = -0.5*(||z_j[n]||^2 - B2_CENTER); biasp cols are
            # a2_i[m] + B2_CENTER per (pair, m-block); sqpos is the
            # positive-pair squared distances.

            # sum_n exp(SHIFT - d) accumulators, one col per (pair, m-block)
            sacc = persist.tile([128, 3 * MB], F32, tag="sacc")
            dpos = persist.tile([128, 3 * MB], F32, tag="dpos")

            # ---- main loop ----
            with (
                tc.tile_pool(name="mpsum", bufs=2, space="PSUM") as mpsum,
                tc.tile_pool(name="dtiles", bufs=6) as dpool,
            ):
                prev_act = None

                def chain(si):
                    nonlocal prev_act
                    if prev_act is not None:
                        add_dep_helper(si.ins, prev_act.ins, sync=True,
                                       reason="act-order")
                    prev_act = si
                    return si

                PHASE_BLKS = 2
                for kk in range(0, MB, PHASE_BLKS):
                    pend = []
                    for k in range(kk, kk + PHASE_BLKS):
                        for p, (i, j) in enumerate(PAIRS):
                            dt = dpool.tile([128, B], FP16, tag="d", name="d")
                            pend.append((dt, p, k))
                            lhs = blks[i][:, k * 128:(k + 1) * 128]
                            for q in range(NQ):
                                ps = mpsum.tile([128, NCHUNK], F32, tag="mm", name="mm")
                                for s in range(NCHUNK // 512):
                                    n0 = q * NCHUNK + s * 512
                                    nc.tensor.matmul(ps[:, s * 512:(s + 1) * 512],
                                                     lhs, ztc[j][q][:, s * 512:(s + 1) * 512],
                                                     start=True, stop=False)
                                    nc.tensor.matmul(ps[:, s * 512:(s + 1) * 512],
                                                     ones_bf_row[0:1, :],
                                                     b2half[j][0:1, n0:n0 + 512],
                                                     start=False, stop=True)
                                # ACT in = -2*(dot - 0.5*(b2-c)) + (a2+c) = a2+b2-2dot
                                chain(nc.scalar.activation(
                                    dt[:, q * NCHUNK:(q + 1) * NCHUNK], ps[:],
                                    AF.Sqrt, bias=biasp_t[:, p * MB + k:p * MB + k + 1], scale=-2.0))
                    if kk == 0:
                        # positive-pair sqrt rides the first sqrt-table phase
                        chain(nc.scalar.activation(dpos[:], sqpos[:], AF.Sqrt))
                    for dt, p, k in pend:
                        chain(nc.scalar.activation(dt[:], dt[:], AF.Exp,
                                                   scale=-1.0 / TAU, bias=shift16[:],
                                                   accum_out=sacc[:, p * MB + k:p * MB + k + 1]))

            # ---- epilogue ----
            with (
                tc.tile_pool(name="fin", bufs=1) as fin,
                tc.tile_pool(name="fpsum", bufs=1, space="PSUM") as fpsum,
            ):
                lnacc = fin.tile([128, 3 * MB], F32)
                nc.scalar.activation(lnacc[:], sacc[:], AF.Ln)

                stack = fin.tile([128, 2], F32)
                nc.vector.tensor_reduce(stack[:, 0:1], dpos[:], AX.X, ALU.add)
                nc.vector.tensor_reduce(stack[:, 1:2], lnacc[:], AX.X, ALU.add)
                fp = fpsum.tile([2, 1], F32)
                nc.tensor.matmul(fp[:], stack[:], ones_f32_col[:],
                                 start=True, stop=True)
                osb = fin.tile([2, 1], F32)
                nc.vector.tensor_copy(osb[:], fp[:])
                nc.sync.dma_start(out[:], osb[:])
    return nc


_CACHE = {}


def kernel(z1: np.ndarray, z2: np.ndarray, z3: np.ndarray) -> np.ndarray:
    zs = [np.asarray(z, dtype=np.float32) for z in (z1, z2, z3)]
    zT = [np.ascontiguousarray(z.T).astype(ml_dtypes.bfloat16) for z in zs]

    # Norm/bias prep from the SAME rounded bf16 values the device multiplies,
    # so sq = a2 + b2 - 2*dot stays the exact squared distance of the rounded
    # vectors (O(B*D) host work, ~0.006% of the kernel's FLOPs).
    zTd = [t.astype(np.float64) for t in zT]
    nrm = [(t * t).sum(0) for t in zTd]                      # ||z_v[n]||^2, [B]
    rhs_views = sorted({j for _, j in PAIRS})
    b2h = {j: (-0.5 * (nrm[j] - B2_CENTER)).astype(ml_dtypes.bfloat16)[None, :]
           for j in rhs_views}
    ip = [(zTd[i] * zTd[j]).sum(0) for i, j in PAIRS]        # <zi_n, zj_n>, [B]

    in_maps = []
    for c in range(NCORES):
        m = {f"zt{v}": zT[v] for v in range(3)}
        for v in range(3):
            m[f"blk{v}"] = np.ascontiguousarray(zT[v][:, c * ML:(c + 1) * ML])
        for j in rhs_views:
            m[f"b2h{j}"] = b2h[j]
        r0 = c * ML
        cols_b, cols_s = [], []
        for p, (i, j) in enumerate(PAIRS):
            a2c = nrm[i][r0:r0 + ML].reshape(MB, 128).T      # [128, MB]
            b2c = nrm[j][r0:r0 + ML].reshape(MB, 128).T
            ipc = ip[p][r0:r0 + ML].reshape(MB, 128).T
            cols_b.append(a2c + B2_CENTER)
            cols_s.append(a2c + b2c - 2.0 * ipc)
        m["biaspall"] = np.concatenate(cols_b, axis=1).astype(np.float32)
        m["sqposall"] = np.concatenate(cols_s, axis=1).astype(np.float32)
        in_maps.append(m)

    if "nc" not in _CACHE:
        nc = bacc.Bacc("TRN2", target_bir_lowering=False)
        build(nc)
        nc.finalize()
        _CACHE["nc"] = nc
    nc = _CACHE["nc"]

    # Host-side checksum: the positive-pair term is O(B*D) to compute exactly
    # and exercises the whole device pipeline (DMA, norms, PE, ACT). A
    # transient runtime fault (observed: silent garbage or
    # NRT_EXEC_UNIT_UNRECOVERABLE after a crashed predecessor) fails this
    # gate, in which case we reset the backend and retry.
    zd = [z.astype(np.float64) for z in zs]
    pos_host = sum(float(np.sqrt(((zd[i] - zd[j]) ** 2).sum(1)).mean())
                   for i, j in PAIRS)

    res = None
    for attempt in range(3):
        try:
            res = run_bass_kernel_spmd(nc, in_maps, core_ids=list(range(NCORES)))
            pos_dev = float(sum(r["out"][0, 0] for r in res.results)) / B
            ln_dev = float(sum(r["out"][1, 0] for r in res.results))
            ok = (np.isfinite(pos_dev) and np.isfinite(ln_dev)
                  and abs(pos_dev - pos_host) <= 0.02 * abs(pos_host) + 1e-6)
        except Exception:
            ok = False
        if ok:
            break
        import time
        import jax
        try:
            jax.clear_backends()
        except Exception:
            pass
        time.sleep(10)
    assert res is not None
    _CACHE["last_res"] = res
    pos_sum = float(sum(r["out"][0, 0] for r in res.results))
    ln_sum = float(sum(r["out"][1, 0] for r in res.results))
    pos_loss = pos_sum / B
    neg_loss = ln_sum / B - len(PAIRS) * (EXP_SHIFT + math.log(B))
    loss = (ALPHA * pos_loss + (1.0 - ALPHA) * neg_loss) / len(PAIRS)
    return np.float32(loss)



# revision 3
# speedup vs baseline: 2.0760x; 2.0760x over previous
"""Trainium2 Bass kernel for nn_LpAlignEntropyLoss.

Loss over three views z1,z2,z3 (each [8192,128] f32):
  for each pair (i<j):
    pos += mean_m ||zi_m - zj_m||
    neg += mean_m [ ln(sum_n exp(-d_mn)) - ln(B) ],  d = cdist(zi, zj)
  loss = (0.5*pos + 0.5*neg) / 3

Strategy: shard the 8192 rows across 8 cores (1024 each). Per core:

  PE   : fp8(e4m3) DoubleRow matmuls compute the full squared distance
         q = a2[m] + b2[n] - 2*zi.zj in ONE pass: k-tile 0 carries
         (-sqrt2*zi) x (sqrt2*zj), k-tile 1 carries the norm terms as
         constant-vector outer products (b2 in 3 fp8 rows against ones,
         a2's 3 fp8 rows against a constant rhs), so no separate bias
         matmuls are needed.  0.5 cycles/row -> ~41us.
  ACT  : single Sqrt pass PSUM f32 -> fp16 d tiles (the only PSUM->SBUF
         crossing; ACT is the bottleneck at ~178us).
  DVE  : one 4x tensor_scalar per row computes the fp16 BIT PATTERN of
         exp((S-d)/tau) via the Schraudolph trick (bits16(2^y) is linear
         in y up to a +-3% mantissa wiggle), writing uint16; a second 4x
         tensor_scalar re-reads the same bytes bitcast to fp16 and
         row-accumulates.
  Host : the +-3% wiggle, the fp8 quantization distance shift, the fp16
         rounding and the ACT sqrt-table error are all removed by an
         on-device calibration: sampled q values go through the SAME
         sqrt->exp-bits->reduce pipeline, and the ratio to their exact
         host sums gives one correction scalar per pair.  The positive-
         pair term is O(B*D) and computed exactly on host.
"""

import math

import numpy as np
import ml_dtypes

import concourse.bacc as bacc
import concourse.bass as bass
import concourse.mybir as mybir
import concourse.tile as tile
from concourse.bass_utils import run_bass_kernel_spmd

B, D = 8192, 128
NCORES = 8
ML = B // NCORES          # rows per core (1024)
MB = ML // 128            # m-blocks per core (8)
PAIRS = [(0, 1), (0, 2), (1, 2)]
TAU = 1.0
ALPHA = 0.5
S_SHIFT = 12.0            # exp((S-d)/tau): keeps fp16 bits in [3k, 16k]
LOG2E = float(np.log2(np.e))
C0E = -1024.0 * LOG2E / TAU
C1E = 1024.0 * (S_SHIFT * LOG2E / TAU + 15.0)
CALN = 1024               # calibration columns per pair per core

F32 = mybir.dt.float32
FP16 = mybir.dt.float16
U16 = mybir.dt.uint16
FP8 = mybir.dt.float8e4
AF = mybir.ActivationFunctionType
ALU = mybir.AluOpType
SQ2 = math.sqrt(2.0)

RHS_VIEWS = (1, 2)        # views used as cdist columns
LHS_VIEWS = (0, 1)        # views used as cdist rows
PAIR_LHS = {0: 0, 1: 0, 2: 1}
PAIR_RHS = {0: 1, 1: 2, 2: 2}


def build(nc: bacc.Bacc):
    rhs_in = {j: nc.dram_tensor(f"rhs{j}", [128, 2, B], FP8, kind="ExternalInput")
              for j in RHS_VIEWS}
    lhs_in = {i: nc.dram_tensor(f"lhs{i}", [128, 2, ML], FP8, kind="ExternalInput")
              for i in LHS_VIEWS}
    calq_in = nc.dram_tensor("calq", [128, 3, CALN], F32, kind="ExternalInput")
    out = nc.dram_tensor("out", [128, 27], F32, kind="ExternalOutput")

    with tile.TileContext(nc) as tc:
        with tc.tile_pool(name="persist", bufs=1) as persist:
            rt = {j: persist.tile([128, 2, B], FP8, tag=f"rt{j}", name=f"rt{j}") for j in RHS_VIEWS}
            lt = {i: persist.tile([128, 2, ML], FP8, tag=f"lt{i}", name=f"lt{i}") for i in LHS_VIEWS}
            cq = persist.tile([128, 3, CALN], F32, tag="cq", name="cq")
            sacc = persist.tile([128, 27], F32, tag="sacc", name="sacc")

            # first pair (0,1) needs rhs1 + lhs0 only; load those first
            nc.sync.dma_start(rt[1][:], rhs_in[1][:])
            nc.sync.dma_start(lt[0][:], lhs_in[0][:])
            nc.sync.dma_start(cq[:], calq_in[:])
            nc.sync.dma_start(lt[1][:], lhs_in[1][:])
            nc.sync.dma_start(rt[2][:], rhs_in[2][:])

            with (
                tc.tile_pool(name="mpsum", bufs=2, space="PSUM") as mpsum,
                tc.tile_pool(name="dpool", bufs=3) as dpool,
                tc.tile_pool(name="epool", bufs=2) as epool,
                tc.tile_pool(name="spool", bufs=1) as spool,
            ):
                scr = spool.tile([128, B], FP16, tag="scr", name="scr")

                col = 0
                for p, (i, j) in enumerate(PAIRS):
                    for k in range(MB):
                        dt = dpool.tile([128, B], FP16, tag="d", name="d")
                        lhsT = lt[i][:, :, k * 128:(k + 1) * 128]
                        for c4 in range(4):
                            ps = mpsum.tile([128, 2048], F32, tag="mm", name="mm")
                            for s in range(4):
                                n0 = c4 * 2048 + s * 512
                                nc.tensor.matmul(
                                    ps[:, s * 512:(s + 1) * 512], lhsT,
                                    rt[j][:, :, n0:n0 + 512],
                                    start=True, stop=True,
                                    perf_mode=mybir.MatmulPerfMode.DoubleRow)
                            nc.scalar.activation(
                                dt[:, c4 * 2048:(c4 + 1) * 2048], ps[:], AF.Sqrt)
                        eb = epool.tile([128, B], U16, tag="e", name="e")
                        nc.vector.tensor_scalar(eb[:], dt[:], C0E, C1E,
                                                ALU.mult, ALU.add)
                        nc.vector.tensor_scalar(scr[:], eb[:].bitcast(FP16),
                                                1.0, 0.0, ALU.mult, ALU.add,
                                                accum_out=sacc[:, col:col + 1])
                        col += 1

                # calibration: same sqrt -> exp-bits -> reduce pipeline on
                # sampled q values (ACT reads SBUF f32 here).
                for p in range(3):
                    dcal = dpool.tile([128, CALN], FP16, tag="dcal", name="dcal")
                    nc.scalar.activation(dcal[:], cq[:, p, :], AF.Sqrt)
                    ebc = epool.tile([128, CALN], U16, tag="ec", name="ec")
                    nc.vector.tensor_scalar(ebc[:], dcal[:], C0E, C1E,
                                            ALU.mult, ALU.add)
                    nc.vector.tensor_scalar(scr[:, 0:CALN], ebc[:].bitcast(FP16),
                                            1.0, 0.0, ALU.mult, ALU.add,
                                            accum_out=sacc[:, 24 + p:24 + p + 1])

            nc.sync.dma_start(out[:], sacc[:])
    return nc


def _q8(a):
    return np.asarray(a, dtype=np.float32).astype(ml_dtypes.float8_e4m3)


def _decomp3(v, first_half=False):
    """Decompose f64 vector v into 3 fp8 rows (r1[*2 if first_half] + r2 + r3)."""
    f64 = np.float64
    if first_half:
        r1 = _q8(v / 2)
        rem = v - 2.0 * r1.astype(f64)
    else:
        r1 = _q8(v)
        rem = v - r1.astype(f64)
    r2 = _q8(rem)
    rem = rem - r2.astype(f64)
    r3 = _q8(rem)
    resid = rem - r3.astype(f64)
    return r1, r2, r3, resid


_CACHE = {}


def kernel(z1: np.ndarray, z2: np.ndarray, z3: np.ndarray) -> np.ndarray:
    f64 = np.float64
    zs = [np.asarray(z, dtype=np.float32) for z in (z1, z2, z3)]
    zT = [np.ascontiguousarray(z.T) for z in zs]            # [128, 8192] f32
    zT64 = [t.astype(f64) for t in zT]

    # fp8 quantizations actually fed to the PE
    rhs0 = {j: _q8(SQ2 * zT[j]) for j in RHS_VIEWS}
    lhs0 = {i: _q8(-SQ2 * zT[i]) for i in LHS_VIEWS}
    eff_r = {j: rhs0[j].astype(f64) / SQ2 for j in RHS_VIEWS}
    eff_l = {i: lhs0[i].astype(f64) / -SQ2 for i in LHS_VIEWS}

    a2 = {i: (eff_l[i] ** 2).sum(0) for i in LHS_VIEWS}     # [8192] f64
    b2 = {j: (eff_r[j] ** 2).sum(0) for j in RHS_VIEWS}

    # aux fp8 rows; device-exact norm sums include the tiny fp8 residual
    g1, g2, g3 = {}, {}, {}
    a2_dev = {}
    for i in LHS_VIEWS:
        g1[i], g2[i], g3[i], res = _decomp3(a2[i], first_half=True)
        a2_dev[i] = a2[i] - res
    h1, h2, h3 = {}, {}, {}
    b2_dev = {}
    for j in RHS_VIEWS:
        h1[j], h2[j], h3[j], res = _decomp3(b2[j])
        b2_dev[j] = b2[j] - res

    rhs_tiles = {}
    for j in RHS_VIEWS:
        k1 = np.zeros((128, B), dtype=ml_dtypes.float8_e4m3)
        k1[0, :] = h1[j]; k1[1, :] = h2[j]; k1[2, :] = h3[j]
        k1[3, :] = 2.0;   k1[4, :] = 1.0;   k1[5, :] = 1.0
        rhs_tiles[j] = np.ascontiguousarray(np.stack([rhs0[j], k1], axis=1))

    lhs_tiles = {}
    for i in LHS_VIEWS:
        k1 = np.zeros((128, B), dtype=ml_dtypes.float8_e4m3)
        k1[0, :] = 1.0; k1[1, :] = 1.0; k1[2, :] = 1.0
        k1[3, :] = g1[i]; k1[4, :] = g2[i]; k1[5, :] = g3[i]
        lhs_tiles[i] = np.ascontiguousarray(np.stack([lhs0[i], k1], axis=1))

    # exact norms/dots of the ORIGINAL f32 inputs (f64 accumulation)
    nrm_true = [(t * t).sum(0) for t in zT64]

    # ---- positive-pair term: exact on host, O(B*D) ----
    pos_loss = sum(
        float(np.sqrt(np.maximum(
            nrm_true[i] + nrm_true[j] - 2.0 * (zT64[i] * zT64[j]).sum(0), 0.0)).mean())
        for i, j in PAIRS)

    # ---- calibration samples: device q-hat vs exact exp sums ----
    rng = np.random.default_rng(12345)
    NS = NCORES * CALN  # samples per pair (one [128, CALN/128... ] slice per core)
    calqs = []          # per core: [128, 3, CALN] f32
    true_sums = np.zeros((NCORES, 3))
    for p, (i, j) in enumerate(PAIRS):
        mi = rng.integers(0, B, size=128 * NS // 128 * 1)  # NS samples
        mi = rng.integers(0, B, size=NS)
        nj = rng.integers(0, B, size=NS)
        dot_eff = (eff_l[i][:, mi] * eff_r[j][:, nj]).sum(0)
        qhat = (a2_dev[i][mi] + b2_dev[j][nj] - 2.0 * dot_eff)
        dot_true = (zT64[i][:, mi] * zT64[j][:, nj]).sum(0)
        d_true = np.sqrt(np.maximum(
            nrm_true[i][mi] + nrm_true[j][nj] - 2.0 * dot_true, 0.0))
        ev = np.exp((S_SHIFT - d_true) / TAU)
        for c in range(NCORES):
            sl = slice(c * CALN, (c + 1) * CALN)
            if p == 0:
                calqs.append(np.zeros((128, 3, CALN), dtype=np.float32))
            calqs[c][:, p, :] = np.float32(qhat[sl])[None, :]
            true_sums[c, p] = 128.0 * ev[sl].sum()

    in_maps = []
    for c in range(NCORES):
        m = {f"rhs{j}": rhs_tiles[j] for j in RHS_VIEWS}
        for i in LHS_VIEWS:
            m[f"lhs{i}"] = np.ascontiguousarray(lhs_tiles[i][:, :, c * ML:(c + 1) * ML])
        m["calq"] = calqs[c]
        in_maps.append(m)

    if "nc" not in _CACHE:
        nc = bacc.Bacc("TRN2", target_bir_lowering=False)
        build(nc)
        nc.finalize()
        _CACHE["nc"] = nc
    nc = _CACHE["nc"]

    res = None
    for attempt in range(4):
        try:
            res = run_bass_kernel_spmd(nc, in_maps, core_ids=list(range(NCORES)))
            outs = [r["out"] for r in res.results]
            cal_dev = np.array([[o[:, 24 + p].sum() for p in range(3)] for o in outs])
            ratios = cal_dev / true_sums
            ok = (np.all(np.isfinite(ratios)) and np.all(ratios > 0.7)
                  and np.all(ratios < 1.5)
                  and all(np.all(np.isfinite(o)) and np.all(o[:, :24] > 0)
                          for o in outs))
        except Exception:
            ok = False
        if ok:
            break
        import time
        import jax
        try:
            jax.clear_backends()
        except Exception:
            pass
        time.sleep(8)
    assert res is not None
    _CACHE["last_res"] = res

    outs = [r["out"].astype(f64) for r in res.results]
    cal_dev = np.array([[o[:, 24 + p].sum() for p in range(3)] for o in outs])
    R = cal_dev.sum(0) / true_sums.sum(0)          # per-pair wiggle ratio

    neg_loss = 0.0
    for p in range(3):
        svals = np.concatenate([o[:, p * MB:(p + 1) * MB].reshape(-1) for o in outs])
        lse = np.log(svals) - math.log(R[p]) - S_SHIFT / TAU
        neg_loss += float(lse.mean()) - math.log(B)

    loss = (ALPHA * pos_loss + (1.0 - ALPHA) * neg_loss) / len(PAIRS)
    return np.float32(loss)


# revision 5
# speedup vs baseline: 2.1536x; 1.0374x over previous
"""Trainium2 Bass kernel for nn_LpAlignEntropyLoss.

Loss over three views z1,z2,z3 (each [8192,128] f32):
  for each pair (i<j):
    pos += mean_m ||zi_m - zj_m||
    neg += mean_m [ ln(sum_n exp(-d_mn)) - ln(B) ],  d = cdist(zi, zj)
  loss = (0.5*pos + 0.5*neg) / 3

Strategy: shard the 8192 rows across 8 cores (1024 each). Per core:

  PE   : fp8(e4m3) DoubleRow matmuls compute the full squared distance
         q = a2[m] + b2[n] - 2*zi.zj in ONE pass: k-tile 0 carries
         (-sqrt2*zi) x (sqrt2*zj), k-tile 1 carries the norm terms as
         constant-vector outer products (b2 in 3 fp8 rows against ones,
         a2's 3 fp8 rows against a constant rhs), so no separate bias
         matmuls are needed.  0.5 cycles/row -> ~41us.
  ACT  : single Sqrt pass PSUM f32 -> fp16 d tiles (the only PSUM->SBUF
         crossing; ACT is the bottleneck at ~178us).
  DVE  : one 4x tensor_scalar per row computes the fp16 BIT PATTERN of
         exp((S-d)/tau) via the Schraudolph trick (bits16(2^y) is linear
         in y up to a +-3% mantissa wiggle), writing uint16; a second 4x
         tensor_scalar re-reads the same bytes bitcast to fp16 and
         row-accumulates.
  Host : the +-3% wiggle, the fp8 quantization distance shift, the fp16
         rounding and the ACT sqrt-table error are all removed by an
         on-device calibration: sampled q values go through the SAME
         sqrt->exp-bits->reduce pipeline, and the ratio to their exact
         host sums gives one correction scalar per pair.  The positive-
         pair term is O(B*D) and computed exactly on host.
"""

import math

import numpy as np
import ml_dtypes

import concourse.bacc as bacc
import concourse.bass as bass
import concourse.mybir as mybir
import concourse.tile as tile
from concourse.bass_utils import run_bass_kernel_spmd

B, D = 8192, 128
NCORES = 8
ML = B // NCORES          # rows per core (1024)
MB = ML // 128            # m-blocks per core (8)
PAIRS = [(0, 1), (0, 2), (1, 2)]
TAU = 1.0
ALPHA = 0.5
S_SHIFT = 12.0            # exp((S-d)/tau): keeps fp16 bits in [3k, 16k]
LOG2E = float(np.log2(np.e))
C0E = -1024.0 * LOG2E / TAU
C1E = 1024.0 * (S_SHIFT * LOG2E / TAU + 15.0)
CALN = 512                # calibration columns per pair per core

F32 = mybir.dt.float32
FP16 = mybir.dt.float16
U16 = mybir.dt.uint16
FP8 = mybir.dt.float8e4
AF = mybir.ActivationFunctionType
ALU = mybir.AluOpType
SQ2 = math.sqrt(2.0)

RHS_VIEWS = (1, 2)        # views used as cdist columns
LHS_VIEWS = (0, 1)        # views used as cdist rows
PAIR_LHS = {0: 0, 1: 0, 2: 1}
PAIR_RHS = {0: 1, 1: 2, 2: 2}


def build(nc: bacc.Bacc):
    rhs_in = {j: nc.dram_tensor(f"rhs{j}", [128, 2, B], FP8, kind="ExternalInput")
              for j in RHS_VIEWS}
    lhs_in = {i: nc.dram_tensor(f"lhs{i}", [128, 2, ML], FP8, kind="ExternalInput")
              for i in LHS_VIEWS}
    calq_in = nc.dram_tensor("calq", [128, 3, CALN], F32, kind="ExternalInput")
    out = nc.dram_tensor("out", [128, 32], F32, kind="ExternalOutput")

    with tile.TileContext(nc) as tc:
        with tc.tile_pool(name="persist", bufs=1) as persist:
            rt = {j: persist.tile([128, 2, B], FP8, tag=f"rt{j}", name=f"rt{j}") for j in RHS_VIEWS}
            lt = {i: persist.tile([128, 2, ML], FP8, tag=f"lt{i}", name=f"lt{i}") for i in LHS_VIEWS}
            cq = persist.tile([128, 3, CALN], F32, tag="cq", name="cq")
            sacc = persist.tile([128, 32], F32, tag="sacc", name="sacc")

            # first pair (0,1) needs lhs0 + leading rhs1 columns; load those
            # first so the PE/ACT pipeline starts ~5us earlier.
            nc.sync.dma_start(lt[0][:], lhs_in[0][:])
            for cdma in range(4):
                nc.sync.dma_start(rt[1][:, :, cdma * 2048:(cdma + 1) * 2048],
                                  rhs_in[1][:, :, cdma * 2048:(cdma + 1) * 2048])
            nc.sync.dma_start(cq[:], calq_in[:])
            nc.sync.dma_start(lt[1][:], lhs_in[1][:])
            for cdma in range(4):
                nc.sync.dma_start(rt[2][:, :, cdma * 2048:(cdma + 1) * 2048],
                                  rhs_in[2][:, :, cdma * 2048:(cdma + 1) * 2048])

            with (
                tc.tile_pool(name="mpsum", bufs=2, space="PSUM") as mpsum,
                tc.tile_pool(name="dpool", bufs=3) as dpool,
                tc.tile_pool(name="epool", bufs=2) as epool,
                tc.tile_pool(name="spool", bufs=1) as spool,
            ):
                scr = spool.tile([128, B], FP16, tag="scr", name="scr")

                def calib_jobs():
                    # same sqrt -> exp-bits -> reduce pipeline on sampled q
                    # values (ACT reads SBUF f32 here).
                    for p in range(3):
                        dcal = dpool.tile([128, CALN], FP16, tag="dcal", name="dcal")
                        nc.scalar.activation(dcal[:], cq[:, p, :], AF.Sqrt)
                        ebc = epool.tile([128, CALN], U16, tag="ec", name="ec")
                        nc.vector.tensor_scalar(ebc[:], dcal[:], C0E, C1E,
                                                ALU.mult, ALU.add)
                        nc.vector.tensor_scalar(scr[:, 0:CALN], ebc[:].bitcast(FP16),
                                                1.0, 0.0, ALU.mult, ALU.add,
                                                accum_out=sacc[:, 24 + p:24 + p + 1])

                col = 0
                njobs = len(PAIRS) * MB
                for p, (i, j) in enumerate(PAIRS):
                    for k in range(MB):
                        last = (col == njobs - 1)
                        dt = dpool.tile([128, B], FP16, tag="d", name="d")
                        lhsT = lt[i][:, :, k * 128:(k + 1) * 128]
                        for c4 in range(4):
                            ps = mpsum.tile([128, 2048], F32, tag="mm", name="mm")
                            for s in range(4):
                                n0 = c4 * 2048 + s * 512
                                nc.tensor.matmul(
                                    ps[:, s * 512:(s + 1) * 512], lhsT,
                                    rt[j][:, :, n0:n0 + 512],
                                    start=True, stop=True,
                                    perf_mode=mybir.MatmulPerfMode.DoubleRow)
                            nc.scalar.activation(
                                dt[:, c4 * 2048:(c4 + 1) * 2048], ps[:], AF.Sqrt)
                            if last:
                                # drain the final row chunk-by-chunk so the DVE
                                # tail overlaps the last ACT chunks
                                sl = slice(c4 * 2048, (c4 + 1) * 2048)
                                eb = epool.tile([128, 2048], U16, tag="el", name="el")
                                nc.vector.tensor_scalar(eb[:], dt[:, sl], C0E, C1E,
                                                        ALU.mult, ALU.add)
                                nc.vector.tensor_scalar(
                                    scr[:, sl], eb[:].bitcast(FP16),
                                    1.0, 0.0, ALU.mult, ALU.add,
                                    accum_out=sacc[:, 27 + c4:28 + c4])
                        if not last:
                            eb = epool.tile([128, B], U16, tag="e", name="e")
                            nc.vector.tensor_scalar(eb[:], dt[:], C0E, C1E,
                                                    ALU.mult, ALU.add)
                            nc.vector.tensor_scalar(scr[:], eb[:].bitcast(FP16),
                                                    1.0, 0.0, ALU.mult, ALU.add,
                                                    accum_out=sacc[:, col:col + 1])
                        col += 1
                        if col == 1:
                            calib_jobs()

            nc.sync.dma_start(out[:], sacc[:])
    return nc


def _q8(a):
    return np.asarray(a, dtype=np.float32).astype(ml_dtypes.float8_e4m3)


def _decomp3(v, first_half=False):
    """Decompose f64 vector v into 3 fp8 rows (r1[*2 if first_half] + r2 + r3)."""
    f64 = np.float64
    if first_half:
        r1 = _q8(v / 2)
        rem = v - 2.0 * r1.astype(f64)
    else:
        r1 = _q8(v)
        rem = v - r1.astype(f64)
    r2 = _q8(rem)
    rem = rem - r2.astype(f64)
    r3 = _q8(rem)
    resid = rem - r3.astype(f64)
    return r1, r2, r3, resid


_CACHE = {}


def kernel(z1: np.ndarray, z2: np.ndarray, z3: np.ndarray) -> np.ndarray:
    f64 = np.float64
    zs = [np.asarray(z, dtype=np.float32) for z in (z1, z2, z3)]
    zT = [np.ascontiguousarray(z.T) for z in zs]            # [128, 8192] f32
    zT64 = [t.astype(f64) for t in zT]

    # fp8 quantizations actually fed to the PE
    rhs0 = {j: _q8(SQ2 * zT[j]) for j in RHS_VIEWS}
    lhs0 = {i: _q8(-SQ2 * zT[i]) for i in LHS_VIEWS}
    eff_r = {j: rhs0[j].astype(f64) / SQ2 for j in RHS_VIEWS}
    eff_l = {i: lhs0[i].astype(f64) / -SQ2 for i in LHS_VIEWS}

    a2 = {i: (eff_l[i] ** 2).sum(0) for i in LHS_VIEWS}     # [8192] f64
    b2 = {j: (eff_r[j] ** 2).sum(0) for j in RHS_VIEWS}

    # aux fp8 rows; device-exact norm sums include the tiny fp8 residual
    g1, g2, g3 = {}, {}, {}
    a2_dev = {}
    for i in LHS_VIEWS:
        g1[i], g2[i], g3[i], res = _decomp3(a2[i], first_half=True)
        a2_dev[i] = a2[i] - res
    h1, h2, h3 = {}, {}, {}
    b2_dev = {}
    for j in RHS_VIEWS:
        h1[j], h2[j], h3[j], res = _decomp3(b2[j])
        b2_dev[j] = b2[j] - res

    rhs_tiles = {}
    for j in RHS_VIEWS:
        k1 = np.zeros((128, B), dtype=ml_dtypes.float8_e4m3)
        k1[0, :] = h1[j]; k1[1, :] = h2[j]; k1[2, :] = h3[j]
        k1[3, :] = 2.0;   k1[4, :] = 1.0;   k1[5, :] = 1.0
        rhs_tiles[j] = np.ascontiguousarray(np.stack([rhs0[j], k1], axis=1))

    lhs_tiles = {}
    for i in LHS_VIEWS:
        k1 = np.zeros((128, B), dtype=ml_dtypes.float8_e4m3)
        k1[0, :] = 1.0; k1[1, :] = 1.0; k1[2, :] = 1.0
        k1[3, :] = g1[i]; k1[4, :] = g2[i]; k1[5, :] = g3[i]
        lhs_tiles[i] = np.ascontiguousarray(np.stack([lhs0[i], k1], axis=1))

    # exact norms/dots of the ORIGINAL f32 inputs (f64 accumulation)
    nrm_true = [(t * t).sum(0) for t in zT64]

    # ---- positive-pair term: exact on host, O(B*D) ----
    pos_loss = sum(
        float(np.sqrt(np.maximum(
            nrm_true[i] + nrm_true[j] - 2.0 * (zT64[i] * zT64[j]).sum(0), 0.0)).mean())
        for i, j in PAIRS)

    # ---- calibration samples: device q-hat vs exact exp sums ----
    rng = np.random.default_rng(12345)
    NS = NCORES * CALN  # samples per pair (one [128, CALN/128... ] slice per core)
    calqs = []          # per core: [128, 3, CALN] f32
    true_sums = np.zeros((NCORES, 3))
    for p, (i, j) in enumerate(PAIRS):
        mi = rng.integers(0, B, size=128 * NS // 128 * 1)  # NS samples
        mi = rng.integers(0, B, size=NS)
        nj = rng.integers(0, B, size=NS)
        dot_eff = (eff_l[i][:, mi] * eff_r[j][:, nj]).sum(0)
        qhat = (a2_dev[i][mi] + b2_dev[j][nj] - 2.0 * dot_eff)
        dot_true = (zT64[i][:, mi] * zT64[j][:, nj]).sum(0)
        d_true = np.sqrt(np.maximum(
            nrm_true[i][mi] + nrm_true[j][nj] - 2.0 * dot_true, 0.0))
        ev = np.exp((S_SHIFT - d_true) / TAU)
        for c in range(NCORES):
            sl = slice(c * CALN, (c + 1) * CALN)
            if p == 0:
                calqs.append(np.zeros((128, 3, CALN), dtype=np.float32))
            calqs[c][:, p, :] = np.float32(qhat[sl])[None, :]
            true_sums[c, p] = 128.0 * ev[sl].sum()

    in_maps = []
    for c in range(NCORES):
        m = {f"rhs{j}": rhs_tiles[j] for j in RHS_VIEWS}
        for i in LHS_VIEWS:
            m[f"lhs{i}"] = np.ascontiguousarray(lhs_tiles[i][:, :, c * ML:(c + 1) * ML])
        m["calq"] = calqs[c]
        in_maps.append(m)

    if "nc" not in _CACHE:
        nc = bacc.Bacc("TRN2", target_bir_lowering=False)
        build(nc)
        nc.finalize()
        _CACHE["nc"] = nc
    nc = _CACHE["nc"]

    res = None
    for attempt in range(4):
        try:
            res = run_bass_kernel_spmd(nc, in_maps, core_ids=list(range(NCORES)))
            outs = [r["out"] for r in res.results]
            cal_dev = np.array([[o[:, 24 + p].sum() for p in range(3)] for o in outs])
            ratios = cal_dev / true_sums
            ok = (np.all(np.isfinite(ratios)) and np.all(ratios > 0.7)
                  and np.all(ratios < 1.5)
                  and all(np.all(np.isfinite(o[:, :31])) and np.all(o[:, :23] > 0)
                          and np.all(o[:, 27:31] > 0) for o in outs))
        except Exception:
            ok = False
        if ok:
            break
        import time
        import jax
        try:
            jax.clear_backends()
        except Exception:
            pass
        time.sleep(8)
    assert res is not None
    _CACHE["last_res"] = res

    outs = [r["out"].astype(f64) for r in res.results]
    cal_dev = np.array([[o[:, 24 + p].sum() for p in range(3)] for o in outs])
    R = cal_dev.sum(0) / true_sums.sum(0)          # per-pair wiggle ratio


    # device col 23 (last row-job) was drained in 4 chunks into cols 27..30
    outs = [np.concatenate([o[:, :23], (o[:, 27:31].sum(1))[:, None]], axis=1)
            for o in outs]
    neg_loss = 0.0
    for p in range(3):
        svals = np.concatenate([o[:, p * MB:(p + 1) * MB].reshape(-1) for o in outs])
        lse = np.log(svals) - math.log(R[p]) - S_SHIFT / TAU
        neg_loss += float(lse.mean()) - math.log(B)

    loss = (ALPHA * pos_loss + (1.0 - ALPHA) * neg_loss) / len(PAIRS)
    return np.float32(loss)
